# revision 18
# baseline (speedup 1.0000x reference)
"""DCGRU cell on 8 Trainium2 NeuronCores (Bass/Tile).

Decomposition
-------------
reference computes, with adj2 = adj + I, d_inv = 1/rowsum(adj2),
adj_mx = (adj2 * d_inv[:, None]).T:

    hop:  x_out = adj_mx @ x_in = adj2^T @ (d_inv * x_in)

Bass matmul computes out = lhsT.T @ rhs with lhsT stationary. The
row-normalization d_inv (and a 4096x range scale) is folded into the
adjacency on the host, so BOTH hop operands are fp8 and every hop runs
in DoubleRow perf mode: one instruction contracts TWO 128-row j-blocks
(lhsT = x pair [128, 2, 128], rhs = adj pair [128, 2, 512]) at the fp8
double rate. DoubleRow requires the stationary free size per k-tile to
be 32/64/128, so the F=66 features are zero-padded to 128 (stream time
is set by the moving free size, so the pad costs nothing there; the
padded PSUM rows 66..127 are never read).

Scales: adj_s = fp8(adj2 * d_inv * 4096); hop inputs are fp8(x) for
x ~ N(0,1)-scale operands (x0, xc) and fp8(64 * x1) for the small
first-hop outputs (x1 ~ 0.012 std would underflow fp8 otherwise). The
scales divide back out in the PSUM evacuation.

Sharding: node dim across 8 cores. Each core keeps its adj2 column
shard [8192, 1024] SBUF-resident in fp8 and reuses it for all 4 hops.
After each of hops 1-3 the thin x is re-replicated with an fp8
AllGather, split in two 512-node halves so the collective for half A
overlaps hop compute of half B.

DMA-instruction count is the scheduling constraint (~0.6us of queue
issue per dma_start regardless of size), so every transfer is batched:
the adjacency shard moves in 8 large DMAs (column-half major, so
hop1's half-0 sweep only waits for half the shard), y0 arrives
feature-padded in one contiguous DMA, the 6 gate-weight slabs ride in
one packed [67, 576] tensor, and each gathered half lands in a single
contiguous DMA (stage payloads are feature-padded to 128 so the
SBUF-side lines are contiguous).

Gate matmuls: xk rows are ordered q = f*3 + m in the reference; W is
host-permuted into per-hop slabs W[m::3] with the bias riding as a
ones-row of the stationary operand. x0^T (feature-major, bf16) is
prepared on the host.
"""

import sys

if "/opt/trn_rl_repo" not in sys.path:
    sys.path.insert(0, "/opt/trn_rl_repo")

import numpy as np
import ml_dtypes

N = 8192
NCORES = 8
S = N // NCORES          # 1024 nodes per core
D_IN = 2
UNITS = 64
F = D_IN + UNITS         # 66
FP = 128                 # padded feature count for DoubleRow lhsT
JBLK = N // 128          # 64 global node blocks
PAIRS = JBLK // 2        # 32 DoubleRow j-block pairs
NBLK = S // 128          # 8 local node blocks
HB = NBLK // 2           # blocks per gather half
BF = ml_dtypes.bfloat16
F8 = ml_dtypes.float8_e4m3
SA = 4096.0              # adjacency scale (~1/d_inv mean)
S1 = 64.0                # staging scale for hop-1 outputs
WCOL = 3 * 2 * UNITS + 3 * UNITS  # packed weight columns (576)

_CACHE = {}


def _build_and_compile():
    import concourse.bacc as bacc
    import concourse.mybir as mybir
    import concourse.tile as tile
    from concourse import masks

    dt = mybir.dt
    AF = mybir.ActivationFunctionType
    ALU = mybir.AluOpType
    DR = mybir.MatmulPerfMode.DoubleRow
    GROUPS = [list(range(NCORES))]

    nc = bacc.Bacc("TRN2", target_bir_lowering=False, debug=False,
                   num_devices=NCORES)

    adj_d = nc.dram_tensor("adj_s", [128, JBLK * S], dt.float8e4,
                           kind="ExternalInput")
    y0_d = nc.dram_tensor("y0_in", [128, JBLK * FP], dt.float8e4,
                          kind="ExternalInput")
    x0T_d = nc.dram_tensor("x0T_in", [F + 1, S], dt.bfloat16,
                           kind="ExternalInput")
    x0l_d = nc.dram_tensor("x0_loc", [128, NBLK * F], dt.float32,
                           kind="ExternalInput")
    w_d = nc.dram_tensor("w_pack", [F + 1, WCOL], dt.bfloat16,
                         kind="ExternalInput")
    out_d = nc.dram_tensor("out_loc", [128, NBLK * UNITS], dt.float32,
                           kind="ExternalOutput")

    # 3 gathers x 2 halves, feature-padded fp8 payloads
    st_d = [[nc.dram_tensor(f"st{i}_{h}", [128, HB * FP], dt.float8e4)
             for h in range(2)] for i in range(3)]
    gf_d = [[nc.dram_tensor(f"gf{i}_{h}", [NCORES, 128, HB * FP], dt.float8e4,
                            addr_space="Shared") for h in range(2)]
            for i in range(3)]

    with tile.TileContext(nc) as tc:
        with (
            tc.tile_pool(name="pers", bufs=1) as pers,
            tc.tile_pool(name="work", bufs=3) as work,
            tc.tile_pool(name="ps_hop", bufs=2, space="PSUM") as ps_hop,
            tc.tile_pool(name="ps_tr", bufs=2, space="PSUM") as ps_tr,
            tc.tile_pool(name="ps_g", bufs=2, space="PSUM") as ps_g,
        ):
            # ---------- persistent tiles ----------
            # y layout [128, core, block, feat]: gather halves land with a
            # single contiguous DMA; pair p covers (c, r) = (p//4, 2*(p%4))
            ya = pers.tile([128, NCORES, NBLK, FP], dt.float8e4, tag="ya")
            yb = pers.tile([128, NCORES, NBLK, FP], dt.float8e4, tag="yb")

            ident_b = pers.tile([128, 128], dt.bfloat16, tag="ident_b")
            masks.make_identity(nc, ident_b[:])

            # thin inputs on the scalar queue; y0 (hop1 dependency) first
            nc.scalar.dma_start(
                ya[:], y0_d.ap().rearrange("p (c r f) -> p c r f",
                                           r=NBLK, f=FP))
            x0T = pers.tile([F + 1, S], dt.bfloat16, tag="x0T")
            nc.scalar.dma_start(x0T[:], x0T_d[:])
            x0l_sb = pers.tile([128, NBLK, F], dt.float32, tag="x0l")
            nc.scalar.dma_start(x0l_sb[:], x0l_d.ap().rearrange(
                "p (nb f) -> p nb f", f=F))
            wp = pers.tile([F + 1, WCOL], dt.bfloat16, tag="wp")
            nc.scalar.dma_start(wp[:], w_d[:])
            w0 = wp[0:F + 1, 0:128]
            w1 = wp[0:F, 128:256]
            w2 = wp[0:F, 256:384]
            wc0 = wp[0:F + 1, 384:448]
            wc1 = wp[0:F, 448:512]
            wc2 = wp[0:F, 512:576]

            # ---------- resident adjacency shard ----------
            # host pre-blocked [128, jb, S]: each group is one contiguous
            # 8KB line per partition. 8 group DMAs alternating between the
            # sync and gpsimd rings so two DMA engines pull in parallel.
            adj_sb = pers.tile([128, JBLK, S], dt.float8e4, tag="adj")
            GRP = 8
            JG = JBLK // GRP
            with nc.named_scope("adj_load"):
                for g in range(GRP):
                    eng = nc.sync if g % 2 == 0 else nc.gpsimd
                    eng.dma_start(
                        adj_sb[:, g * JG:(g + 1) * JG, :],
                        adj_d[:, g * JG * S:(g + 1) * JG * S].rearrange(
                            "p (jb s) -> p jb s", s=S))

            x1T = pers.tile([F, S], dt.bfloat16, tag="x1T")
            x2T = pers.tile([F, S], dt.bfloat16, tag="x2T")
            xcT = pers.tile([F + 1, S], dt.bfloat16, tag="xcT")
            x1cT = pers.tile([F, S], dt.bfloat16, tag="x1cT")
            x2cT = pers.tile([F, S], dt.bfloat16, tag="x2cT")
            # partition access must start at a quadrant boundary: set rows
            # 64..66 to 1.0; rows 64,65 are overwritten by the transposes.
            nc.gpsimd.memset(xcT[64:F + 1, :], 1.0)

            stA = pers.tile([128, NBLK, FP], dt.float8e4, tag="stA")
            stB = pers.tile([128, NBLK, FP], dt.float8e4, tag="stB")
            stC = pers.tile([128, NBLK, FP], dt.float8e4, tag="stC")
            for st8 in (stA, stB, stC):
                nc.gpsimd.memset(st8[:, :, F:FP], 0.0)
            gates_sb = pers.tile([128, NBLK, 2 * UNITS], dt.float32, tag="gates")
            xc_sb = pers.tile([128, NBLK, F], dt.bfloat16, tag="xc")
            out_sb = pers.tile([128, NBLK, UNITS], dt.float32, tag="out")

            def pair_half_order():
                """pair order: all j-pairs of gather-half 0 first, then
                half 1 (half 1's collective may still be in flight)."""
                seq = []
                for h in range(2):
                    for c in range(NCORES):
                        for k in range(HB // 2):
                            seq.append(c * (NBLK // 2) + h * (HB // 2) + k)
                return seq

            def hop(y_tile, evac, order=None, jb_major=False):
                """x_out^T[:, core cols] = DoubleRow pair-matmuls.

                ci-major (default): sweep all pairs for output columns
                0:512, evacuate + gather that half while the 512:1024
                sweep runs. jb-major (hop1): interleave both column
                halves per pair so the sweep tracks the DMA'd j-groups;
                both halves finish with the last group."""
                seq = order if order is not None else list(range(PAIRS))
                rp = NBLK // 2
                if jb_major:
                    phs = [ps_hop.tile([FP, 512], dt.float32, tag="ph",
                                       name=f"ph{ci}")
                           for ci in range(2)]
                    for i, p in enumerate(seq):
                        c, r = p // rp, 2 * (p % rp)
                        for ci in range(2):
                            nc.tensor.matmul(
                                phs[ci][:], y_tile[:, c, r:r + 2, :],
                                adj_sb[:, 2 * p:2 * p + 2,
                                       ci * 512:(ci + 1) * 512],
                                start=(i == 0), stop=(i == PAIRS - 1),
                                perf_mode=DR)
                    for ci in range(2):
                        evac(ci, phs[ci])
                    return
                for ci in range(2):
                    ph = ps_hop.tile([FP, 512], dt.float32, tag="ph")
                    for i, p in enumerate(seq):
                        c, r = p // rp, 2 * (p % rp)
                        nc.tensor.matmul(
                            ph[:], y_tile[:, c, r:r + 2, :],
                            adj_sb[:, 2 * p:2 * p + 2,
                                   ci * 512:(ci + 1) * 512],
                            start=(i == 0), stop=(i == PAIRS - 1),
                            perf_mode=DR)
                    evac(ci, ph)

            def stage_half(xT_tile, stage, st, gf, h, scale):
                """transpose local node blocks of half h into fp8, gather."""
                for k in range(HB):
                    nb = h * HB + k
                    pt = ps_tr.tile([128, F], dt.bfloat16, tag="pt2")
                    nc.tensor.transpose(
                        pt[:], xT_tile[0:F, nb * 128:(nb + 1) * 128],
                        ident_b[0:F, 0:F])
                    nc.scalar.activation(stage[:, nb, 0:F], pt[:], AF.Copy,
                                         scale=scale)
                nc.scalar.dma_start(
                    st.ap().rearrange("p (nb f) -> p nb f", f=FP),
                    stage[:, h * HB:(h + 1) * HB, :])
                nc.gpsimd.collective_compute(
                    "AllGather", ALU.bypass, replica_groups=GROUPS,
                    ins=[st[:]], outs=[gf[:]])

            def load_half(gf, y_next, h):
                """gathered half h -> y blocks (c, h*HB + k), split over
                the three DMA-capable queues to parallelize descriptor
                generation (the source is 512B-fragmented per rank)."""
                src = gf.ap().rearrange("c p (k f) -> p c k f", f=FP)
                for eng, c0, c1 in ((nc.gpsimd, 0, 3), (nc.scalar, 3, 6),
                                    (nc.sync, 6, 8)):
                    eng.dma_start(
                        y_next[:, c0:c1, h * HB:(h + 1) * HB, :],
                        src[:, c0:c1, :, :])

            # ---------- gconv 1 (gates r, u) ----------
            with nc.named_scope("hop1"):
                def evac1(ci, ph):
                    nc.scalar.activation(
                        x1T[:, ci * 512:(ci + 1) * 512], ph[0:F, :], AF.Copy,
                        scale=1.0 / SA)
                    stage_half(x1T, stA, st_d[0][ci], gf_d[0][ci], ci,
                               scale=S1)
                hop(ya, evac1, jb_major=True)
            with nc.named_scope("gather1"):
                for h in range(2):
                    load_half(gf_d[0][h], yb, h)

            def gates_block(nb):
                pg = ps_g.tile([128, 2 * UNITS], dt.float32, tag="pg")
                sl = slice(nb * 128, (nb + 1) * 128)
                nc.tensor.matmul(pg[:], x0T[:, sl], w0, start=True, stop=False)
                nc.tensor.matmul(pg[:], x1T[:, sl], w1, start=False, stop=False)
                nc.tensor.matmul(pg[:], x2T[:, sl], w2, start=False,
                                 stop=True)
                nc.scalar.activation(gates_sb[:, nb, :], pg[:], AF.Sigmoid)
                # x_c = [inp | r * hx]
                nc.vector.tensor_copy(xc_sb[:, nb, 0:D_IN],
                                      x0l_sb[:, nb, 0:D_IN])
                nc.vector.tensor_mul(xc_sb[:, nb, D_IN:F],
                                     gates_sb[:, nb, 0:UNITS],
                                     x0l_sb[:, nb, D_IN:F])
                nc.vector.tensor_copy(stB[:, nb, 0:F], xc_sb[:, nb, :])
                pt = ps_tr.tile([F, 128], dt.bfloat16, tag="pt")
                nc.tensor.transpose(pt[:], xc_sb[:, nb, :], ident_b[:])
                nc.vector.tensor_copy(xcT[0:F, nb * 128:(nb + 1) * 128], pt[:])

            def evac2(ci, ph):
                nc.vector.scalar_tensor_tensor(
                    x2T[:, ci * 512:(ci + 1) * 512], ph[0:F, :],
                    2.0 / (SA * S1), x0T[0:F, ci * 512:(ci + 1) * 512],
                    op0=ALU.mult, op1=ALU.subtract)
                for nb in range(ci * HB, (ci + 1) * HB):
                    gates_block(nb)
                nc.scalar.dma_start(
                    st_d[1][ci].ap().rearrange("p (nb f) -> p nb f", f=FP),
                    stB[:, ci * HB:(ci + 1) * HB, :])
                nc.gpsimd.collective_compute(
                    "AllGather", ALU.bypass, replica_groups=GROUPS,
                    ins=[st_d[1][ci][:]], outs=[gf_d[1][ci][:]])

            with nc.named_scope("hop2"):
                hop(yb, evac2, order=pair_half_order())

            # ---------- gconv 2 (candidate c) ----------
            with nc.named_scope("gather2"):
                for h in range(2):
                    load_half(gf_d[1][h], ya, h)
            with nc.named_scope("hop1c"):
                def evac1c(ci, ph):
                    nc.scalar.activation(
                        x1cT[:, ci * 512:(ci + 1) * 512], ph[0:F, :], AF.Copy,
                        scale=1.0 / SA)
                    stage_half(x1cT, stC, st_d[2][ci], gf_d[2][ci], ci,
                               scale=S1)
                hop(ya, evac1c, order=pair_half_order())
            with nc.named_scope("gather3"):
                for h in range(2):
                    load_half(gf_d[2][h], yb, h)

            def final_block(nb):
                pc = ps_g.tile([128, UNITS], dt.float32, tag="pg")
                sl = slice(nb * 128, (nb + 1) * 128)
                nc.tensor.matmul(pc[:], xcT[:, sl], wc0, start=True, stop=False)
                nc.tensor.matmul(pc[:], x1cT[:, sl], wc1, start=False, stop=False)
                nc.tensor.matmul(pc[:], x2cT[:, sl], wc2, start=False,
                                 stop=True)
                c_sb = work.tile([128, UNITS], dt.float32, tag="c")
                nc.scalar.activation(c_sb[:], pc[:], AF.Tanh)
                # new = c + u * (hx - c)
                t1 = work.tile([128, UNITS], dt.float32, tag="t1")
                nc.vector.tensor_sub(t1[:], x0l_sb[:, nb, D_IN:F], c_sb[:])
                t2 = work.tile([128, UNITS], dt.float32, tag="t2")
                nc.vector.tensor_mul(t2[:], gates_sb[:, nb, UNITS:2 * UNITS],
                                     t1[:])
                nc.vector.tensor_add(out_sb[:, nb, :], c_sb[:], t2[:])

            def evac2c(ci, ph):
                nc.vector.scalar_tensor_tensor(
                    x2cT[:, ci * 512:(ci + 1) * 512], ph[0:F, :],
                    2.0 / (SA * S1), xcT[0:F, ci * 512:(ci + 1) * 512],
                    op0=ALU.mult, op1=ALU.subtract)
                for nb in range(ci * HB, (ci + 1) * HB):
                    final_block(nb)

            with nc.named_scope("hop2c"):
                hop(yb, evac2c, order=pair_half_order())
            nc.scalar.dma_start(
                out_d.ap().rearrange("p (nb u) -> p nb u", u=UNITS), out_sb[:])

    nc.compile()
    return nc


def _get_nc():
    if "nc" not in _CACHE:
        _CACHE["nc"] = _build_and_compile()
    return _CACHE["nc"]


def _host_prep(inputs, hx, adj, w_ru, b_ru, w_c, b_c):
    x0 = np.concatenate(
        [np.asarray(inputs, np.float32).reshape(N, D_IN),
         np.asarray(hx, np.float32).reshape(N, UNITS)], axis=1)
    adj = np.asarray(adj, np.float32)
    w_ru = np.asarray(w_ru, np.float32)
    w_c = np.asarray(w_c, np.float32)
    wpack = np.zeros((F + 1, WCOL), np.float32)
    wpack[:, 0:128] = np.vstack([w_ru[0::3], np.asarray(b_ru, np.float32)[None, :]])
    wpack[0:F, 128:256] = w_ru[1::3]
    wpack[0:F, 256:384] = w_ru[2::3]
    wpack[:, 384:448] = np.vstack([w_c[0::3], np.asarray(b_c, np.float32)[None, :]])
    wpack[0:F, 448:512] = w_c[1::3]
    wpack[0:F, 512:576] = w_c[2::3]
    wpack = wpack.astype(BF)
    # normalized + scaled adjacency: adj_s[j, i] = (adj2 * d_inv)[j, i] * SA
    scale = (SA / (1.0 + adj.sum(axis=1))).astype(np.float32)
    adj_s = (adj * scale[:, None]).astype(F8)
    diag = np.arange(N)
    adj_s[diag, diag] = ((adj[diag, diag] + 1.0) * scale).astype(F8)
    y0_blk = np.zeros((128, JBLK, FP), F8)
    y0_blk[:, :, 0:F] = x0.astype(F8).reshape(JBLK, 128, F).transpose(1, 0, 2)
    y0_blk = np.ascontiguousarray(y0_blk.reshape(128, JBLK * FP))
    ones = np.ones((1, S), np.float32)
    in_maps = []
    for m in range(NCORES):
        sl = slice(m * S, (m + 1) * S)
        in_maps.append({
            "adj_s": np.ascontiguousarray(
                adj_s[:, sl].reshape(JBLK, 128, S).transpose(1, 0, 2)
                .reshape(128, JBLK * S)),
            "y0_in": y0_blk,
            "x0T_in": np.ascontiguousarray(
                np.vstack([x0[sl].T, ones]).astype(BF)),
            "x0_loc": np.ascontiguousarray(
                x0[sl].reshape(NBLK, 128, F).transpose(1, 0, 2).reshape(
                    128, NBLK * F)),
            "w_pack": wpack,
        })
    return in_maps


def _run(in_maps, trace=False):
    from concourse.bass_utils import run_bass_kernel_spmd
    nc = _get_nc()
    res = run_bass_kernel_spmd(nc, in_maps, list(range(NCORES)), trace=trace)
    out = np.concatenate(
        [np.asarray(res.results[m]["out_loc"]).reshape(128, NBLK, UNITS)
         .transpose(1, 0, 2).reshape(S, UNITS) for m in range(NCORES)], axis=0)
    return out.reshape(1, N * UNITS).astype(np.float32), res


def kernel(**inputs):
    in_maps = _host_prep(
        inputs["inputs"], inputs["hx"], inputs["adj"], inputs["w_ru"],
        inputs["b_ru"], inputs["w_c"], inputs["b_c"])
    out, _ = _run(in_maps, trace=False)
    return out


# revision 23
# speedup vs baseline: 1.1759x; 1.1759x over previous
"""DCGRU cell on 8 Trainium2 NeuronCores (Bass/Tile).

Decomposition
-------------
reference computes, with adj2 = adj + I, d_inv = 1/rowsum(adj2),
adj_mx = (adj2 * d_inv[:, None]).T:

    hop:  x_out = adj_mx @ x_in = adj2^T @ (d_inv * x_in)

Bass matmul computes out = lhsT.T @ rhs with lhsT stationary. The
row-normalization d_inv (and a 4096x range scale) is folded into the
adjacency on the host, so BOTH hop operands are fp8 and every hop runs
in DoubleRow perf mode: one instruction contracts TWO 128-row j-blocks
(lhsT = x pair [128, 2, 128], rhs = adj pair [128, 2, 512]) at the fp8
double rate. DoubleRow requires the stationary free size per k-tile to
be 32/64/128, so the F=66 features are zero-padded to 128 (stream time
is set by the moving free size, so the pad costs nothing there; the
padded PSUM rows 66..127 are never read).

Scales: adj_s = fp8(adj2 * d_inv * 4096); hop inputs are fp8(x) for
x ~ N(0,1)-scale operands (x0, xc) and fp8(64 * x1) for the small
first-hop outputs (x1 ~ 0.012 std would underflow fp8 otherwise). The
scales divide back out in the PSUM evacuation.

Sharding: node dim across 8 cores. Each core keeps its adj2 column
shard [8192, 1024] SBUF-resident in fp8 and reuses it for all 4 hops.
After each of hops 1-3 the thin x is re-replicated with an fp8
AllGather, split in two 512-node halves so the collective for half A
overlaps hop compute of half B.

DMA-instruction count is the scheduling constraint (~0.6us of queue
issue per dma_start regardless of size), so every transfer is batched:
the adjacency shard moves in 8 large DMAs (column-half major, so
hop1's half-0 sweep only waits for half the shard), y0 arrives
feature-padded in one contiguous DMA, the 6 gate-weight slabs ride in
one packed [67, 576] tensor, and each gathered half lands in a single
contiguous DMA (stage payloads are feature-padded to 128 so the
SBUF-side lines are contiguous).

Gate matmuls: xk rows are ordered q = f*3 + m in the reference; W is
host-permuted into per-hop slabs W[m::3] with the bias riding as a
ones-row of the stationary operand. x0^T (feature-major, bf16) is
prepared on the host.
"""

import sys

if "/opt/trn_rl_repo" not in sys.path:
    sys.path.insert(0, "/opt/trn_rl_repo")

import numpy as np
import ml_dtypes

N = 8192
NCORES = 8
S = N // NCORES          # 1024 nodes per core
D_IN = 2
UNITS = 64
F = D_IN + UNITS         # 66
FP = 128                 # padded feature count for DoubleRow lhsT
JBLK = N // 128          # 64 global node blocks
PAIRS = JBLK // 2        # 32 DoubleRow j-block pairs
NBLK = S // 128          # 8 local node blocks
HB = NBLK // 2           # blocks per gather half
BF = ml_dtypes.bfloat16
F8 = ml_dtypes.float8_e4m3
SA = 4096.0              # adjacency scale (~1/d_inv mean)
S1 = 64.0                # staging scale for hop-1 outputs
WCOL = 3 * 2 * UNITS + 3 * UNITS  # packed weight columns (576)

_CACHE = {}


def _build_and_compile():
    import concourse.bacc as bacc
    import concourse.mybir as mybir
    import concourse.tile as tile
    from concourse import masks

    dt = mybir.dt
    AF = mybir.ActivationFunctionType
    ALU = mybir.AluOpType
    DR = mybir.MatmulPerfMode.DoubleRow
    GROUPS = [list(range(NCORES))]

    nc = bacc.Bacc("TRN2", target_bir_lowering=False, debug=False,
                   num_devices=NCORES)

    adj_d = nc.dram_tensor("adj_s", [128, JBLK * S], dt.float8e4,
                           kind="ExternalInput")
    y0_d = nc.dram_tensor("y0_in", [128, JBLK * FP], dt.float8e4,
                          kind="ExternalInput")
    x0T_d = nc.dram_tensor("x0T_in", [F + 1, S], dt.bfloat16,
                           kind="ExternalInput")
    x0l_d = nc.dram_tensor("x0_loc", [128, NBLK * F], dt.float32,
                           kind="ExternalInput")
    w_d = nc.dram_tensor("w_pack", [F + 1, WCOL], dt.bfloat16,
                         kind="ExternalInput")
    out_d = nc.dram_tensor("out_loc", [128, NBLK * UNITS], dt.float32,
                           kind="ExternalOutput")

    # 3 gathers x 2 halves, feature-padded fp8 payloads
    st_d = [[nc.dram_tensor(f"st{i}_{h}", [128, HB * FP], dt.float8e4)
             for h in range(2)] for i in range(3)]
    gf_d = [[nc.dram_tensor(f"gf{i}_{h}", [NCORES, 128, HB * FP], dt.float8e4,
                            addr_space="Shared") for h in range(2)]
            for i in range(3)]

    with tile.TileContext(nc) as tc:
        with (
            tc.tile_pool(name="pers", bufs=1) as pers,
            tc.tile_pool(name="work", bufs=3) as work,
            tc.tile_pool(name="ps_hop", bufs=2, space="PSUM") as ps_hop,
            tc.tile_pool(name="ps_tr", bufs=2, space="PSUM") as ps_tr,
            tc.tile_pool(name="ps_g", bufs=2, space="PSUM") as ps_g,
        ):
            # ---------- persistent tiles ----------
            # y layout [128, core, block, feat]: gather halves land with a
            # single contiguous DMA; pair p covers (c, r) = (p//4, 2*(p%4))
            ya = pers.tile([128, NCORES, NBLK, FP], dt.float8e4, tag="ya")
            yb = pers.tile([128, NCORES, NBLK, FP], dt.float8e4, tag="yb")

            ident_b = pers.tile([128, 128], dt.bfloat16, tag="ident_b")
            masks.make_identity(nc, ident_b[:])

            # thin inputs on the scalar queue; y0 (hop1 dependency) first
            nc.scalar.dma_start(
                ya[:], y0_d.ap().rearrange("p (c r f) -> p c r f",
                                           r=NBLK, f=FP))
            x0T = pers.tile([F + 1, S], dt.bfloat16, tag="x0T")
            nc.scalar.dma_start(x0T[:], x0T_d[:])
            x0l_sb = pers.tile([128, NBLK, F], dt.float32, tag="x0l")
            nc.scalar.dma_start(x0l_sb[:], x0l_d.ap().rearrange(
                "p (nb f) -> p nb f", f=F))
            wp = pers.tile([F + 1, WCOL], dt.bfloat16, tag="wp")
            nc.scalar.dma_start(wp[:], w_d[:])
            w0 = wp[0:F + 1, 0:128]
            w1 = wp[0:F, 128:256]
            w2 = wp[0:F, 256:384]
            wc0 = wp[0:F + 1, 384:448]
            wc1 = wp[0:F, 448:512]
            wc2 = wp[0:F, 512:576]

            # ---------- resident adjacency shard ----------
            # host pre-blocked column-half-major [128, half, jb, 512]:
            # every group DMA is one contiguous 8KB line per partition on
            # both sides, and all of column-half 0 (hop sweeps' ci=0)
            # arrives in the first half of the load. Groups alternate
            # between the sync and gpsimd rings so two DMA engines pull
            # in parallel.
            adj_sb = pers.tile([128, 2, JBLK, 512], dt.float8e4, tag="adj")
            GRP = 4
            JG = JBLK // GRP
            with nc.named_scope("adj_load"):
                for half in range(2):
                    for g in range(GRP):
                        eng = nc.sync if g % 2 == 0 else nc.gpsimd
                        off = (half * JBLK + g * JG) * 512
                        eng.dma_start(
                            adj_sb[:, half, g * JG:(g + 1) * JG, :],
                            adj_d[:, off:off + JG * 512].rearrange(
                                "p (jb s) -> p jb s", s=512))

            x1T = pers.tile([F, S], dt.bfloat16, tag="x1T")
            x2T = pers.tile([F, S], dt.bfloat16, tag="x2T")
            xcT = pers.tile([F + 1, S], dt.bfloat16, tag="xcT")
            x1cT = pers.tile([F, S], dt.bfloat16, tag="x1cT")
            x2cT = pers.tile([F, S], dt.bfloat16, tag="x2cT")
            # partition access must start at a quadrant boundary: set rows
            # 64..66 to 1.0; rows 64,65 are overwritten by the transposes.
            nc.gpsimd.memset(xcT[64:F + 1, :], 1.0)

            stA = pers.tile([128, NBLK, FP], dt.float8e4, tag="stA")
            stB = pers.tile([128, NBLK, FP], dt.float8e4, tag="stB")
            stC = pers.tile([128, NBLK, FP], dt.float8e4, tag="stC")
            for st8 in (stA, stB, stC):
                nc.gpsimd.memset(st8[:, :, F:FP], 0.0)
            gates_sb = pers.tile([128, NBLK, 2 * UNITS], dt.float32, tag="gates")
            xc_sb = pers.tile([128, NBLK, F], dt.bfloat16, tag="xc")
            out_sb = pers.tile([128, NBLK, UNITS], dt.float32, tag="out")

            def pair_half_order():
                """pair order: all j-pairs of gather-half 0 first, then
                half 1 (half 1's collective may still be in flight)."""
                seq = []
                for h in range(2):
                    for c in range(NCORES):
                        for k in range(HB // 2):
                            seq.append(c * (NBLK // 2) + h * (HB // 2) + k)
                return seq

            def hop(y_tile, evac, order=None, jb_major=False):
                """x_out^T[:, core cols] = DoubleRow pair-matmuls.

                ci-major (default): sweep all pairs for output columns
                0:512, evacuate + gather that half while the 512:1024
                sweep runs. jb-major (hop1): interleave both column
                halves per pair so the sweep tracks the DMA'd j-groups;
                both halves finish with the last group."""
                seq = order if order is not None else list(range(PAIRS))
                rp = NBLK // 2
                if jb_major:
                    phs = [ps_hop.tile([FP, 512], dt.float32, tag="ph",
                                       name=f"ph{ci}")
                           for ci in range(2)]
                    for i, p in enumerate(seq):
                        c, r = p // rp, 2 * (p % rp)
                        for ci in range(2):
                            nc.tensor.matmul(
                                phs[ci][:], y_tile[:, c, r:r + 2, :],
                                adj_sb[:, ci, 2 * p:2 * p + 2, :],
                                start=(i == 0), stop=(i == PAIRS - 1),
                                perf_mode=DR)
                    for ci in range(2):
                        evac(ci, phs[ci])
                    return
                for ci in range(2):
                    ph = ps_hop.tile([FP, 512], dt.float32, tag="ph")
                    for i, p in enumerate(seq):
                        c, r = p // rp, 2 * (p % rp)
                        nc.tensor.matmul(
                            ph[:], y_tile[:, c, r:r + 2, :],
                            adj_sb[:, ci, 2 * p:2 * p + 2, :],
                            start=(i == 0), stop=(i == PAIRS - 1),
                            perf_mode=DR)
                    evac(ci, ph)

            def stage_half(xT_tile, stage, st, gf, h, scale):
                """transpose local node blocks of half h into fp8, gather."""
                for k in range(HB):
                    nb = h * HB + k
                    pt = ps_tr.tile([128, F], dt.bfloat16, tag="pt2")
                    nc.tensor.transpose(
                        pt[:], xT_tile[0:F, nb * 128:(nb + 1) * 128],
                        ident_b[0:F, 0:F])
                    nc.scalar.activation(stage[:, nb, 0:F], pt[:], AF.Copy,
                                         scale=scale)
                nc.scalar.dma_start(
                    st.ap().rearrange("p (nb f) -> p nb f", f=FP),
                    stage[:, h * HB:(h + 1) * HB, :])
                nc.gpsimd.collective_compute(
                    "AllGather", ALU.bypass, replica_groups=GROUPS,
                    ins=[st[:]], outs=[gf[:]])

            def load_half(gf, y_next, h):
                """gathered half h -> y blocks (c, h*HB + k), split over
                the three DMA-capable queues to parallelize descriptor
                generation (the source is 512B-fragmented per rank)."""
                src = gf.ap().rearrange("c p (k f) -> p c k f", f=FP)
                for eng, c0, c1 in ((nc.scalar, 0, 4), (nc.sync, 4, 8)):
                    eng.dma_start(
                        y_next[:, c0:c1, h * HB:(h + 1) * HB, :],
                        src[:, c0:c1, :, :])

            # ---------- gconv 1 (gates r, u) ----------
            with nc.named_scope("hop1"):
                def evac1(ci, ph):
                    nc.scalar.activation(
                        x1T[:, ci * 512:(ci + 1) * 512], ph[0:F, :], AF.Copy,
                        scale=1.0 / SA)
                    stage_half(x1T, stA, st_d[0][ci], gf_d[0][ci], ci,
                               scale=S1)
                hop(ya, evac1)
            with nc.named_scope("gather1"):
                for h in range(2):
                    load_half(gf_d[0][h], yb, h)

            def gates_block(nb):
                pg = ps_g.tile([128, 2 * UNITS], dt.float32, tag="pg")
                sl = slice(nb * 128, (nb + 1) * 128)
                nc.tensor.matmul(pg[:], x0T[:, sl], w0, start=True, stop=False)
                nc.tensor.matmul(pg[:], x1T[:, sl], w1, start=False, stop=False)
                nc.tensor.matmul(pg[:], x2T[:, sl], w2, start=False,
                                 stop=True)
                nc.scalar.activation(gates_sb[:, nb, :], pg[:], AF.Sigmoid)
                # x_c = [inp | r * hx]
                nc.vector.tensor_copy(xc_sb[:, nb, 0:D_IN],
                                      x0l_sb[:, nb, 0:D_IN])
                nc.vector.tensor_mul(xc_sb[:, nb, D_IN:F],
                                     gates_sb[:, nb, 0:UNITS],
                                     x0l_sb[:, nb, D_IN:F])
                nc.vector.tensor_copy(stB[:, nb, 0:F], xc_sb[:, nb, :])
                pt = ps_tr.tile([F, 128], dt.bfloat16, tag="pt")
                nc.tensor.transpose(pt[:], xc_sb[:, nb, :], ident_b[:])
                nc.vector.tensor_copy(xcT[0:F, nb * 128:(nb + 1) * 128], pt[:])

            def evac2(ci, ph):
                nc.vector.scalar_tensor_tensor(
                    x2T[:, ci * 512:(ci + 1) * 512], ph[0:F, :],
                    2.0 / (SA * S1), x0T[0:F, ci * 512:(ci + 1) * 512],
                    op0=ALU.mult, op1=ALU.subtract)
                for nb in range(ci * HB, (ci + 1) * HB):
                    gates_block(nb)
                nc.scalar.dma_start(
                    st_d[1][ci].ap().rearrange("p (nb f) -> p nb f", f=FP),
                    stB[:, ci * HB:(ci + 1) * HB, :])
                nc.gpsimd.collective_compute(
                    "AllGather", ALU.bypass, replica_groups=GROUPS,
                    ins=[st_d[1][ci][:]], outs=[gf_d[1][ci][:]])

            with nc.named_scope("hop2"):
                hop(yb, evac2, order=pair_half_order())

            # ---------- gconv 2 (candidate c) ----------
            with nc.named_scope("gather2"):
                for h in range(2):
                    load_half(gf_d[1][h], ya, h)
            with nc.named_scope("hop1c"):
                def evac1c(ci, ph):
                    nc.scalar.activation(
                        x1cT[:, ci * 512:(ci + 1) * 512], ph[0:F, :], AF.Copy,
                        scale=1.0 / SA)
                    stage_half(x1cT, stC, st_d[2][ci], gf_d[2][ci], ci,
                               scale=S1)
                hop(ya, evac1c, order=pair_half_order())
            with nc.named_scope("gather3"):
                for h in range(2):
                    load_half(gf_d[2][h], yb, h)

            def final_block(nb):
                pc = ps_g.tile([128, UNITS], dt.float32, tag="pg")
                sl = slice(nb * 128, (nb + 1) * 128)
                nc.tensor.matmul(pc[:], xcT[:, sl], wc0, start=True, stop=False)
                nc.tensor.matmul(pc[:], x1cT[:, sl], wc1, start=False, stop=False)
                nc.tensor.matmul(pc[:], x2cT[:, sl], wc2, start=False,
                                 stop=True)
                c_sb = work.tile([128, UNITS], dt.float32, tag="c")
                nc.scalar.activation(c_sb[:], pc[:], AF.Tanh)
                # new = c + u * (hx - c)
                t1 = work.tile([128, UNITS], dt.float32, tag="t1")
                nc.vector.tensor_sub(t1[:], x0l_sb[:, nb, D_IN:F], c_sb[:])
                t2 = work.tile([128, UNITS], dt.float32, tag="t2")
                nc.vector.tensor_mul(t2[:], gates_sb[:, nb, UNITS:2 * UNITS],
                                     t1[:])
                nc.vector.tensor_add(out_sb[:, nb, :], c_sb[:], t2[:])

            def evac2c(ci, ph):
                nc.vector.scalar_tensor_tensor(
                    x2cT[:, ci * 512:(ci + 1) * 512], ph[0:F, :],
                    2.0 / (SA * S1), xcT[0:F, ci * 512:(ci + 1) * 512],
                    op0=ALU.mult, op1=ALU.subtract)
                for nb in range(ci * HB, (ci + 1) * HB):
                    final_block(nb)

            with nc.named_scope("hop2c"):
                hop(yb, evac2c, order=pair_half_order())
            nc.scalar.dma_start(
                out_d.ap().rearrange("p (nb u) -> p nb u", u=UNITS), out_sb[:])

    nc.compile()
    return nc


def _get_nc():
    if "nc" not in _CACHE:
        _CACHE["nc"] = _build_and_compile()
    return _CACHE["nc"]


def _host_prep(inputs, hx, adj, w_ru, b_ru, w_c, b_c):
    x0 = np.concatenate(
        [np.asarray(inputs, np.float32).reshape(N, D_IN),
         np.asarray(hx, np.float32).reshape(N, UNITS)], axis=1)
    adj = np.asarray(adj, np.float32)
    w_ru = np.asarray(w_ru, np.float32)
    w_c = np.asarray(w_c, np.float32)
    wpack = np.zeros((F + 1, WCOL), np.float32)
    wpack[:, 0:128] = np.vstack([w_ru[0::3], np.asarray(b_ru, np.float32)[None, :]])
    wpack[0:F, 128:256] = w_ru[1::3]
    wpack[0:F, 256:384] = w_ru[2::3]
    wpack[:, 384:448] = np.vstack([w_c[0::3], np.asarray(b_c, np.float32)[None, :]])
    wpack[0:F, 448:512] = w_c[1::3]
    wpack[0:F, 512:576] = w_c[2::3]
    wpack = wpack.astype(BF)
    # normalized + scaled adjacency: adj_s[j, i] = (adj2 * d_inv)[j, i] * SA
    scale = (SA / (1.0 + adj.sum(axis=1))).astype(np.float32)
    adj_s = (adj * scale[:, None]).astype(F8)
    diag = np.arange(N)
    adj_s[diag, diag] = ((adj[diag, diag] + 1.0) * scale).astype(F8)
    y0_blk = np.zeros((128, JBLK, FP), F8)
    y0_blk[:, :, 0:F] = x0.astype(F8).reshape(JBLK, 128, F).transpose(1, 0, 2)
    y0_blk = np.ascontiguousarray(y0_blk.reshape(128, JBLK * FP))
    ones = np.ones((1, S), np.float32)
    in_maps = []
    for m in range(NCORES):
        sl = slice(m * S, (m + 1) * S)
        in_maps.append({
            "adj_s": np.ascontiguousarray(
                adj_s[:, sl].reshape(JBLK, 128, 2, 512)
                .transpose(1, 2, 0, 3).reshape(128, JBLK * S)),
            "y0_in": y0_blk,
            "x0T_in": np.ascontiguousarray(
                np.vstack([x0[sl].T, ones]).astype(BF)),
            "x0_loc": np.ascontiguousarray(
                x0[sl].reshape(NBLK, 128, F).transpose(1, 0, 2).reshape(
                    128, NBLK * F)),
            "w_pack": wpack,
        })
    return in_maps


def _run(in_maps, trace=False):
    from concourse.bass_utils import run_bass_kernel_spmd
    nc = _get_nc()
    res = run_bass_kernel_spmd(nc, in_maps, list(range(NCORES)), trace=trace)
    out = np.concatenate(
        [np.asarray(res.results[m]["out_loc"]).reshape(128, NBLK, UNITS)
         .transpose(1, 0, 2).reshape(S, UNITS) for m in range(NCORES)], axis=0)
    return out.reshape(1, N * UNITS).astype(np.float32), res


def kernel(**inputs):
    in_maps = _host_prep(
        inputs["inputs"], inputs["hx"], inputs["adj"], inputs["w_ru"],
        inputs["b_ru"], inputs["w_c"], inputs["b_c"])
    out, _ = _run(in_maps, trace=False)
    return out
